# revision 41
# baseline (speedup 1.0000x reference)
"""Trainium2 kernel for nn_EquivariantConvExp (dense_cnn, memory-bound).

The reference applies, per block i, a truncated conv-exponential
exp(Conv_i) to z of shape [B, 1, 1, 2].  A 3x3 "same" conv on a 1x2 image
only ever uses the middle filter row, so Conv_i acts on each sample pair
(z0, z1) as the 2x2 matrix A_i = [[f11, f12], [f10, f11]] built from the
expanded filter's middle row.  The truncated exponential is the 2x2 matrix
E_i = sum_{k=0}^{10} A_i^k / k!, and the chain over 8 blocks collapses to a
single 2x2 matrix M = E_0 @ E_1 @ ... @ E_7 with z_out = z_in @ M^T.
log|det| is the per-sample constant c = -2 * sum_i f11_i.

The device kernel is therefore a streaming 2x2 affine map over 4M pairs,
data-parallel across 8 NeuronCores (batch sharding, no collectives), plus a
constant fill for the logdet plane.  Per core: 4 MiB in + 6 MiB out at the
~360 GB/s per-NC HBM cap -> ~27 us of streaming, plus ~7 us of fixed NRT/
preamble cost; measured ~37-43 us end-to-end.

Structure per core:
  - SP HWDGE ring: four 1 MiB contiguous loads back-to-back, then four
    1 MiB stores, each issued as its tile's compute lands (paced by
    dve_sem).  The logdet plane rides the ACT HWDGE ring so it streams
    concurrently with the loads.
  - ACT: per 1024-col chunk, the two cross terms tb = xo*m01 and
    tc = xe*m10 (stride-2 reads; ScalarE has no fp32 fast mode, ~1.2
    ns/elem regardless of stride).
  - DVE: per chunk, the two fused scalar_tensor_tensor ops
    oe = xe*m00 + tb and oo = xo*m11 + tc, writing the interleaved
    output tile in place.
  - Pool: unused (its elementwise compute is ~100x too slow for strided
    work, SWDGE adds exit-drain cost, and its memset is slow enough to
    land pool_sem on the critical path — DVE fills the logdet tile
    during its idle lead-in instead).

Raw bass (manual semaphores) rather than Tile: this walrus build encodes at
most one attached sync-wait per instruction, which Tile's auto-generated
multi-wait sync_info exceeds; standalone wait_ge instructions are the
proven-good encoding.  Two hard-won correctness rules: (1) cross-engine
handoffs must signal via drain().then_inc — a bare op.then_inc can fire
before the producer's SBUF writes are visible to a DMA reader; (2) each
load gets its own semaphore — the 16 SDMA engine-slots increment
independently and skew across outstanding transfers, so a shared counter's
cumulative value does not prove an individual transfer landed.
"""

import numpy as np

N_CORES = 8
P = 128                      # SBUF partitions
B = 4194304                  # total batch rows
BS = B // N_CORES            # rows per core
FLAT = BS * 2                # fp32 elements per core
W = FLAT // P                # total columns per partition
T = 2048                     # cols per DMA tile (1 MiB transfers)
NT = W // T                  # DMA tiles (loads and stores each)
CW = 1024                    # cols per compute chunk
K = T // CW                  # compute chunks per DMA tile
NCH = NT * K                 # total compute chunks
CH = CW // 2                 # pairs per partition per chunk
LD_W = BS // P               # logdet elements per partition

N_TERMS = 10

# Set by the test harness to capture a profile; LAST_RESULT holds the
# BassKernelResults of the most recent device run.
TRACE = False
LAST_RESULT = None


def _collapse(weights, basis):
    """Reduce (weights, basis) to the 2x2 matrix M and logdet constant c."""
    w = np.asarray(weights, dtype=np.float64)
    bs = np.asarray(basis, dtype=np.float64)
    n_blocks = w.shape[0]
    M = np.eye(2)
    c = 0.0
    for i in range(n_blocks):
        f = np.einsum("n,nhw->hw", w[i], bs[i, :, 0, 0])
        A = np.array([[f[1, 1], f[1, 2]], [f[1, 0], f[1, 1]]])
        E = np.eye(2)
        term = np.eye(2)
        for k in range(1, N_TERMS + 1):
            term = term @ A / k
            E = E + term
        M = M @ E
        c -= 2.0 * f[1, 1]
    return M, c


def _build(nc_cls, mybir, M, c):
    f32 = mybir.dt.float32
    AF = mybir.ActivationFunctionType
    OP = mybir.AluOpType

    m00 = float(np.float32(M[0, 0]))
    m01 = float(np.float32(M[0, 1]))
    m10 = float(np.float32(M[1, 0]))
    m11 = float(np.float32(M[1, 1]))
    cf = float(np.float32(c))

    nc = nc_cls()
    xin = nc.declare_dram_parameter("x", [BS, 2], f32, isOutput=False)
    zout = nc.declare_dram_parameter("z", [BS, 2], f32, isOutput=True)
    ldout = nc.declare_dram_parameter("ld", [BS], f32, isOutput=True)

    # Tile-major views: DMA tile n is the contiguous flat range
    # [n*P*T, (n+1)*P*T), laid out [P, T] with partition p owning a
    # contiguous T-element run.  Every transfer is a fully sequential
    # DRAM range with 16 KiB-per-partition chunks (the fastest observed
    # DMA pattern); compute sub-chunks tiles at CW columns, so pipeline
    # granularity stays fine without shrinking transfers.
    xv = xin[:].flatten().rearrange("(n p t) -> n p t", p=P, t=T)
    zv = zout[:].flatten().rearrange("(n p t) -> n p t", p=P, t=T)
    ldv = ldout[:].rearrange("(p t) -> p t", p=P)

    from contextlib import ExitStack

    with ExitStack() as ctx:
        xbuf = ctx.enter_context(nc.sbuf_tensor([P, W], f32))
        obuf = ctx.enter_context(nc.sbuf_tensor([P, W], f32))
        tbuf = ctx.enter_context(nc.sbuf_tensor([P, W], f32))
        lt = ctx.enter_context(nc.sbuf_tensor([P, LD_W], f32))
        # One semaphore per load: the 16 SDMA engine-slots increment
        # independently and can skew across outstanding transfers, so a
        # shared counter's cumulative value does not imply transfer n
        # fully landed.  A per-transfer sem hitting 16 does.
        in_sems = [
            ctx.enter_context(nc.semaphore(f"in_sem{n}")) for n in range(NT)
        ]
        out_sem = ctx.enter_context(nc.semaphore("out_sem"))
        act_sem = ctx.enter_context(nc.semaphore("act_sem"))
        pool_sem = ctx.enter_context(nc.semaphore("pool_sem"))
        dve_sem = ctx.enter_context(nc.semaphore("dve_sem"))
        block = ctx.enter_context(nc.Block())

        @block.sync
        def _(sync):
            # SP ring: read stream back-to-back, then the write stream as
            # each tile's compute lands.  The logdet plane rides the ACT
            # HWDGE ring concurrently.
            for n in range(NT):
                sync.dma_start(
                    out=xbuf[:, n * T : (n + 1) * T], in_=xv[n]
                ).then_inc(in_sems[n], 16)
            for n in range(NT):
                sync.wait_ge(dve_sem, n + 1)
                sync.dma_start(
                    out=zv[n], in_=obuf[:, n * T : (n + 1) * T]
                ).then_inc(out_sem, 16)
            sync.wait_ge(out_sem, 16 * (NT + 1))

        @block.scalar
        def _(s):
            # logdet store on the ACT ring (overlaps the loads), then the
            # per-tile cross terms tb = xo*m01, tc = xe*m10 (half-width,
            # stride-2 reads).
            s.wait_ge(pool_sem, 1)
            s.dma_start(out=ldv[:, :], in_=lt[:]).then_inc(out_sem, 16)
            for j in range(NCH):
                if j % K == 0:
                    s.wait_ge(in_sems[j // K], 16)
                xt = xbuf[:, j * CW : (j + 1) * CW]
                xe = xt[:, 0::2]
                xo = xt[:, 1::2]
                tb0 = j * CW
                s.activation(
                    tbuf[:, tb0 : tb0 + CH], xo, AF.Copy,
                    bias=0.0, scale=m01,
                )
                s.activation(
                    tbuf[:, tb0 + CH : tb0 + 2 * CH], xe, AF.Copy,
                    bias=0.0, scale=m10,
                )
                s.drain().then_inc(act_sem, 1)

        @block.vector
        def _(v):
            # DVE is idle until ACT's first tile lands (~13 us) — do the
            # logdet fill here (2x memset mode, ~2.1 us) so pool_sem fires
            # ~9 us, before L0 lands, keeping the ld trigger off ACT's
            # critical path.  (GpSimd's memset takes 3.5 us and lands the
            # sem right at the L0 edge.)
            v.memset(lt[:], cf)
            v.drain().then_inc(pool_sem, 1)
            # oe = xe*m00 + tb ; oo = xo*m11 + tc  (fused on DVE).
            # Drain-inc once per TILE (not per chunk): stores only need
            # per-tile granularity, and the saved DVE drains (~0.55 us
            # each) tighten the pipeline under HBM contention.
            for j in range(NCH):
                v.wait_ge(act_sem, j + 1)
                xt = xbuf[:, j * CW : (j + 1) * CW]
                xe = xt[:, 0::2]
                xo = xt[:, 1::2]
                ot = obuf[:, j * CW : (j + 1) * CW]
                oe = ot[:, 0::2]
                oo = ot[:, 1::2]
                tb0 = j * CW
                v.scalar_tensor_tensor(
                    oe, xe, m00, tbuf[:, tb0 : tb0 + CH],
                    OP.mult, OP.add,
                )
                v.scalar_tensor_tensor(
                    oo, xo, m11, tbuf[:, tb0 + CH : tb0 + 2 * CH],
                    OP.mult, OP.add,
                )
                if (j + 1) % K == 0:
                    v.drain().then_inc(dve_sem, 1)

    return nc


def _run_device(x, M, c):
    import concourse.bass as bass
    import concourse.mybir as mybir
    from concourse.bass_utils import run_bass_kernel_spmd

    nc = _build(bass.Bass, mybir, M, c)

    xs = np.ascontiguousarray(x.reshape(N_CORES, BS, 2))
    in_maps = [{"x": xs[i]} for i in range(N_CORES)]
    res = run_bass_kernel_spmd(
        nc, in_maps, core_ids=list(range(N_CORES)), trace=TRACE
    )
    global LAST_RESULT
    LAST_RESULT = res
    z = np.concatenate([res.results[i]["z"] for i in range(N_CORES)], axis=0)
    ld = np.concatenate([res.results[i]["ld"] for i in range(N_CORES)], axis=0)
    return z, ld


def kernel(x, weights, basis):
    x = np.ascontiguousarray(np.asarray(x, dtype=np.float32))
    M, c = _collapse(weights, basis)
    if x.shape != (B, 2):
        # Shape drift guard: same math on host.
        z = (x.astype(np.float64) @ M.T).astype(np.float32)
        ld = np.full((x.shape[0],), np.float32(c), dtype=np.float32)
        return z, ld
    return _run_device(x, M, c)
